# revision 30
# baseline (speedup 1.0000x reference)
"""Distributed KNN online evaluator kernel for 8 trn2 NeuronCores.

Device side (SPMD over 8 cores, bank sharded over N):
  - bf16 matmul sim tiles (queries stationary) -> f32 PSUM, 26 slots of
    2048 cols (13 bank groups x 2 query chunks), psum ring 2
  - split-slot evac, engine-balanced, all PSUM-bank aligned:
      ACT: copy psum cols [0:aw]   -> bf16 stage (aw = 1536, or 1024 for
           two groups to balance ACT vs DVE)
      DVE: blockmax-8 reduce of psum cols [aw:2048] (blocks of 8)
           + 2-level TT fold tree of the staged aw (blocks of 4,
           stride aw/4)
  - bank fully resident in SBUF (53KB/partition), loaded once
  - batched blockmax stores (8 DMAs)

Host side:
  - one-shot drill-down: select blocks whose blockmax could contain a
    global top-K sim, recompute those sims exactly in f32, take top-K;
    verified sound (unselected blocks provably below the top-K threshold
    within MARGIN) with an expansion fallback loop
  - class votes with inf weights degenerate to membership -> output is
    [voted classes asc, unvoted classes asc] per query
"""

import numpy as np
import ml_dtypes

import concourse.bass as bass
import concourse.mybir as mybir
from concourse.bass_utils import run_bass_kernel_spmd

BF16 = ml_dtypes.bfloat16

N_CORES = 8
B = 256  # queries
D = 128  # feature dim
N_TOTAL = 200000
N_SHARD = N_TOTAL // N_CORES  # 25000
NBG = 13  # bank groups
GW = 2048  # group width (cols) = one psum slot
NCOL = NBG * GW  # padded shard width 26624
NSLOT = 2 * NBG  # 26 compute slots: slot j -> group j//2, chunk j%2
S_GROUPS = set()  # groups with a 1024/1024 split; empty: uniform 1536/512
#                   (DVE-heavy slots stall the 2-deep psum ring, so uniform
#                   ACT-leaning slots win despite higher ACT busy)
AW_G = [1024 if g in S_GROUPS else 1536 for g in range(NBG)]
A_OUT_G = [aw // 4 for aw in AW_G]  # A-blocks per group (stride aw/4)
B_OUT_G = [(GW - aw) // 8 for aw in AW_G]  # B-blocks per group
AOFF_G = np.concatenate([[0], np.cumsum(A_OUT_G)]).astype(int)
BOFF_G = np.concatenate([[0], np.cumsum(B_OUT_G)]).astype(int)
A_COLS = int(AOFF_G[-1])  # 4736 per chunk
B_COLS = int(BOFF_G[-1])  # 960 per chunk
AW_MAX = 1536
K = 200
NUM_CLASSES = 1000
MARGIN = 0.75  # device blockmax fuzz bound vs exact f32 sim (bf16 inputs;
#                worst-case approx 0.33 input-quant + 0.35 bf16 out round;
#                measured max over 51M sims: 0.125)

_NC_CACHE = None


def _build_nc():
    nc = bass.Bass("TRN2", target_bir_lowering=False, debug=False,
                   num_devices=N_CORES)
    qT = nc.dram_tensor("qT", [D, B], mybir.dt.bfloat16,
                        kind="ExternalInput").ap()
    bankT = nc.dram_tensor("bankT", [D, NCOL], mybir.dt.bfloat16,
                           kind="ExternalInput").ap()
    bmA = nc.dram_tensor("bmA", [B, A_COLS], mybir.dt.bfloat16,
                         kind="ExternalOutput").ap()
    bmB = nc.dram_tensor("bmB", [B, B_COLS], mybir.dt.bfloat16,
                         kind="ExternalOutput").ap()

    with (
        nc.sbuf_tensor([D, B], mybir.dt.bfloat16) as qs,
        nc.sbuf_tensor([D, NCOL], mybir.dt.bfloat16) as banks,  # resident
        nc.psum_tensor([128, 2 * GW], mybir.dt.float32) as psum,  # ring 2
        nc.sbuf_tensor([128, 3 * AW_MAX], mybir.dt.bfloat16) as stage,
        nc.sbuf_tensor([128, 2 * (AW_MAX // 2)], mybir.dt.bfloat16) as l1,
        nc.sbuf_tensor([128, 2 * A_COLS], mybir.dt.bfloat16) as obufA,
        nc.sbuf_tensor([128, 2 * B_COLS], mybir.dt.bfloat16) as obufB,
        nc.semaphore() as warm_sem,
        nc.semaphore() as dma_sem,
        nc.semaphore() as st_sem,   # store-DMA completions (nobody waits)
        nc.semaphore() as mm0_sem,  # slot 0 cols [0:1536] done (ACT start)
        nc.semaphore() as mm_sem,   # +1 per slot (on its 4th matmul)
        nc.semaphore() as evacA,    # +1 per ACT slot copy
        nc.semaphore() as bsem,     # +1 per DVE direct reduce
        nc.semaphore() as fsem,     # +1 per DVE fold (slot fully done)
        nc.Block() as block,
    ):
        @block.sync
        def _(sync):
            sync.dma_start(qs[:], qT).then_inc(dma_sem, 16)
            for k in range(4):  # group 0 in quarters: earliest first matmul
                sync.dma_start(banks[:, k * 512:(k + 1) * 512],
                               bankT[:, k * 512:(k + 1) * 512]
                               ).then_inc(dma_sem, 16)
            for g in range(1, NBG):
                sync.dma_start(banks[:, g * GW:(g + 1) * GW],
                               bankT[:, g * GW:(g + 1) * GW]
                               ).then_inc(dma_sem, 16)
            # batched stores; thresholds count slots in j = 2g+c order
            a7, b7 = int(AOFF_G[7]), int(BOFF_G[7])
            a12, b12 = int(AOFF_G[12]), int(BOFF_G[12])
            stores = [
                (bmA, 0, 0, a7, fsem, 13), (bmA, 1, 0, a7, fsem, 14),
                (bmB, 0, 0, b7, bsem, 13), (bmB, 1, 0, b7, bsem, 14),
                (bmA, 0, a7, a12, fsem, 23), (bmA, 1, a7, a12, fsem, 24),
                (bmA, 0, a12, A_COLS, fsem, 25),
                (bmB, 0, b7, B_COLS, bsem, 25),
                (bmA, 1, a12, A_COLS, fsem, 26),
                (bmB, 1, b7, B_COLS, bsem, 26),
            ]
            for dram, c, lo, hi, sem, thr in stores:
                ob, w = (obufA, A_COLS) if dram is bmA else (obufB, B_COLS)
                sync.wait_ge(sem, thr)
                sync.dma_start(dram[c * 128:(c + 1) * 128, lo:hi],
                               ob[:, c * w + lo:c * w + hi]
                               ).then_inc(st_sem, 16)

        @block.tensor
        def _(tensor):
            for j in range(NSLOT):
                g, c = j // 2, j % 2
                s = (j % 2) * GW
                if j >= 2:  # psum ring: slot j-2 fully evacuated
                    tensor.wait_ge(evacA, j - 1)
                    tensor.wait_ge(bsem, j - 1)
                for k in range(4):
                    if g == 0 and c == 0:
                        tensor.wait_ge(dma_sem, 16 * (k + 2))
                    elif k == 0:
                        tensor.wait_ge(dma_sem, 16 * (g + 5))
                    mm = tensor.matmul(
                        psum[:, s + k * 512: s + (k + 1) * 512],
                        lhsT=qs[:, c * 128:(c + 1) * 128],
                        rhs=banks[:, g * GW + k * 512: g * GW + (k + 1) * 512],
                        start=True, stop=True)
                    if j == 0 and k == 2:
                        mm.then_inc(mm0_sem, 1)
                    if k == 3:
                        mm.then_inc(mm_sem, 1)

        @block.scalar
        def _(scalar):
            # warmup: load the ACT copy table before real work arrives
            scalar.wait_ge(warm_sem, 1)
            scalar.copy(stage[:, 0:1], stage[:, 1:2])
            for j in range(NSLOT):
                aw = AW_G[j // 2]
                if j >= 3:  # stage ring 3: fold of slot j-3 done
                    scalar.wait_ge(fsem, j - 2)
                if j == 0:
                    scalar.wait_ge(mm0_sem, 1)  # cols 0:1536 ready
                else:
                    scalar.wait_ge(mm_sem, j + 1)
                s = (j % 2) * GW
                ss = (j % 3) * AW_MAX
                scalar.copy(stage[:, ss:ss + aw],
                            psum[:, s:s + aw]).then_inc(evacA, 1)

        @block.vector
        def _(vector):
            MAX = mybir.AluOpType.max
            vector.memset(stage[:, 0:2], 0).then_inc(warm_sem, 1)
            for j in range(NSLOT):
                g, c = j // 2, j % 2
                aw = AW_G[g]
                ah = aw // 2
                aout = aw // 4
                bout = B_OUT_G[g]
                s = (j % 2) * GW
                ss = (j % 3) * AW_MAX
                ls = (j % 2) * (AW_MAX // 2)
                # direct blockmax-8 of psum cols [aw:GW]
                vector.wait_ge(mm_sem, j + 1)
                vector.tensor_reduce(
                    out=obufB[:, c * B_COLS + BOFF_G[g]:
                              c * B_COLS + BOFF_G[g] + bout],
                    in_=psum[:, s + aw:s + GW].rearrange(
                        "p (b w) -> p b w", w=8),
                    axis=mybir.AxisListType.X,
                    op=MAX,
                ).then_inc(bsem, 1)
                # 2-level fold tree of the staged A cols
                vector.wait_ge(evacA, j + 1)
                vector.tensor_tensor(
                    out=l1[:, ls:ls + ah], in0=stage[:, ss:ss + ah],
                    in1=stage[:, ss + ah:ss + aw], op=MAX)
                vector.tensor_tensor(
                    out=obufA[:, c * A_COLS + AOFF_G[g]:
                              c * A_COLS + AOFF_G[g] + aout],
                    in0=l1[:, ls:ls + aout],
                    in1=l1[:, ls + aout:ls + ah],
                    op=MAX).then_inc(fsem, 1)
    return nc


def _get_nc():
    global _NC_CACHE
    if _NC_CACHE is None:
        _NC_CACHE = _build_nc()
    return _NC_CACHE


def _run_device(query_feature, feature_bank, trace=False):
    qT = np.ascontiguousarray(query_feature.astype(np.float32).T
                              ).astype(BF16)  # [128, 256]
    in_maps = []
    for i in range(N_CORES):
        shard = feature_bank[i * N_SHARD:(i + 1) * N_SHARD].astype(np.float32)
        bt = np.zeros((D, NCOL), dtype=BF16)
        bt[:, :N_SHARD] = np.ascontiguousarray(shard.T).astype(BF16)
        in_maps.append({"qT": qT, "bankT": bt})
    nc = _get_nc()
    res = run_bass_kernel_spmd(nc, in_maps, list(range(N_CORES)), trace=trace)
    bmA = np.stack([res.results[i]["bmA"].astype(np.float32)
                    for i in range(N_CORES)])  # [8, 256, A_COLS]
    bmB = np.stack([res.results[i]["bmB"].astype(np.float32)
                    for i in range(N_CORES)])  # [8, 256, B_COLS]
    return (bmA, bmB), res


def _block_rows():
    """Local bank-row sets per (core-local) block, sentinel-padded to 8.

    Block order per core: A-blocks (AOFF_G layout) then B-blocks.
    A block (g, t) covers cols g*GW + {t, t+ao, t+2*ao, t+3*ao}, ao = aw/4;
    B block (g, u) covers cols g*GW + aw + 8u .. +8.
    Rows >= N_SHARD (padding) get sentinel (mapped to N_TOTAL later).
    """
    arows = []
    brows = []
    for g in range(NBG):
        aw = AW_G[g]
        ao = aw // 4
        t = np.arange(ao)[:, None]
        arows.append(g * GW + t + ao * np.arange(4)[None, :])
        u = np.arange(B_OUT_G[g])[:, None]
        brows.append(g * GW + aw + 8 * u + np.arange(8)[None, :])
    arows = np.concatenate(arows, axis=0)  # [A_COLS, 4]
    arows = np.concatenate(
        [arows, np.full((arows.shape[0], 4), N_SHARD)], axis=1)
    brows = np.concatenate(brows, axis=0)  # [B_COLS, 8]
    return np.concatenate([arows, brows], axis=0)  # [A_COLS+B_COLS, 8]


def _host_topk(bm, query_feature, feature_bank):
    """bm = (bmA, bmB): [8, 256, A_COLS], [8, 256, B_COLS] f32 device
    blockmaxima. Returns top-K indices [B, K] into the full bank, matching
    f32 jax top_k semantics."""
    bmA, bmB = bm
    q = query_feature.astype(np.float32)
    fb = feature_bank.astype(np.float32)

    lrows = _block_rows()  # [NBLK_CORE, 8] local rows (sentinel N_SHARD pad)
    nbc = lrows.shape[0]  # blocks per core
    grow = np.where(lrows < N_SHARD,
                    lrows[None, :, :] + (np.arange(N_CORES) * N_SHARD
                                         )[:, None, None],
                    N_TOTAL).reshape(-1, 8)  # [8*nbc, 8]

    bm_core = np.concatenate([bmA, bmB], axis=2)  # [8, 256, nbc]
    bm_flat = np.ascontiguousarray(
        bm_core.transpose(1, 0, 2).reshape(B, N_CORES * nbc))
    nblk = bm_flat.shape[1]

    # threshold: every block whose blockmax could reach the top-K values
    bm200 = -np.partition(-bm_flat, K - 1, axis=1)[:, K - 1]
    nsel = (bm_flat >= (bm200 - 2 * MARGIN)[:, None]).sum(axis=1)
    nb = int(nsel.max())

    topk_idx = np.empty((B, K), dtype=np.int64)
    pending = np.arange(B)
    while len(pending):
        nb = min(nb, nblk)
        part = np.argpartition(-bm_flat[pending], nb - 1, axis=1)
        still = []
        for chunk in range(0, len(pending), 64):
            pc = pending[chunk:chunk + 64]
            nq = len(pc)
            sel = part[chunk:chunk + 64, :nb]
            # gather A-blocks (4 real rows) and B-blocks (8) separately
            pairs = sel.ravel()
            pair_q = np.repeat(np.arange(nq), nb)
            isA = (pairs % nbc) < A_COLS
            sims8 = np.full((nq * nb, 8), -np.inf, dtype=np.float32)
            rows8 = grow[pairs]
            for msk, w in ((isA, 4), (~isA, 8)):
                r = rows8[msk, :w]
                s = np.einsum("prd,pd->pr", fb[np.minimum(r, N_TOTAL - 1)],
                              q[pc][pair_q[msk]], optimize=True)
                s[r == N_TOTAL] = -np.inf
                sims8[msk, :w] = s
            sims = sims8.reshape(nq, nb * 8)
            rows = rows8.reshape(nq, nb * 8)
            for i, b in enumerate(pc):
                o = np.lexsort((rows[i], -sims[i]))[:K]
                tK = sims[i][o[-1]]
                if nb >= nblk:
                    topk_idx[b] = rows[i][o]
                    continue
                unsel = bm_flat[b][part[chunk + i, nb:]].max()
                if unsel + MARGIN < tK:
                    topk_idx[b] = rows[i][o]
                else:
                    still.append(b)
        pending = np.array(still, dtype=np.int64)
        nb *= 2
    return topk_idx


def _labels_to_output(topk_idx, target_bank):
    tb = np.asarray(target_bank).astype(np.int64)
    mask = np.zeros((B, NUM_CLASSES), dtype=bool)
    mask[np.arange(B)[:, None], tb[topk_idx]] = True
    # voted classes ascending, then unvoted ascending
    return np.argsort(~mask, axis=1, kind="stable").astype(np.int32)


def kernel(query_feature, feature_bank, target_bank):
    query_feature = np.asarray(query_feature)
    feature_bank = np.asarray(feature_bank)
    target_bank = np.asarray(target_bank)
    bm, _ = _run_device(query_feature, feature_bank)
    topk_idx = _host_topk(bm, query_feature, feature_bank)
    return _labels_to_output(topk_idx, target_bank)


# revision 33
# speedup vs baseline: 1.0162x; 1.0162x over previous
"""Distributed KNN online evaluator kernel for 8 trn2 NeuronCores.

Device side (SPMD over 8 cores, bank sharded over N):
  - bf16 matmul sim tiles (queries stationary) -> f32 PSUM, 26 slots of
    2048 cols (13 bank groups x 2 query chunks), psum ring 2
  - split-slot evac, engine-balanced, all PSUM-bank aligned:
      ACT: copy psum cols [0:1536] -> bf16 stage           (~1465 ns)
      DVE: blockmax-8 reduce of psum cols [1536:2048]      ( ~659 ns)
           + 2-level TT fold tree of the staged 1536       ( ~720 ns)
    (mid-bank reduce starts crash the HW; offsets must stay bank aligned)
  - bank fully resident in SBUF (53KB/partition), loaded once
  - batched blockmax stores (8 DMAs)

Host side:
  - one-shot drill-down: select blocks whose blockmax could contain a
    global top-K sim, recompute those sims exactly in f32, take top-K;
    verified sound (unselected blocks provably below the top-K threshold
    within MARGIN) with an expansion fallback loop
  - class votes with inf weights degenerate to membership -> output is
    [voted classes asc, unvoted classes asc] per query
"""

import numpy as np
import ml_dtypes

import concourse.bass as bass
import concourse.mybir as mybir
from concourse.bass_utils import run_bass_kernel_spmd

BF16 = ml_dtypes.bfloat16

N_CORES = 8
B = 256  # queries
D = 128  # feature dim
N_TOTAL = 200000
N_SHARD = N_TOTAL // N_CORES  # 25000
NBG = 13  # bank groups
GW = 2048  # group width (cols) = one psum slot
NCOL = NBG * GW  # padded shard width 26624
NSLOT = 2 * NBG  # 26 compute slots: slot j -> group j//2, chunk j%2
S_GROUPS = set()  # groups with a 1024/1024 split; empty: uniform 1536/512
#                   (DVE-heavy slots stall the 2-deep psum ring, so uniform
#                   ACT-leaning slots win despite higher ACT busy)
AW_G = [1024 if g in S_GROUPS else 1536 for g in range(NBG)]
A_OUT_G = [aw // 4 for aw in AW_G]  # A-blocks per group (stride aw/4)
B_OUT_G = [(GW - aw) // 8 for aw in AW_G]  # B-blocks per group
AOFF_G = np.concatenate([[0], np.cumsum(A_OUT_G)]).astype(int)
BOFF_G = np.concatenate([[0], np.cumsum(B_OUT_G)]).astype(int)
A_COLS = int(AOFF_G[-1])  # 4992 per chunk
B_COLS = int(BOFF_G[-1])  # 832 per chunk
AW_MAX = 1536
K = 200
NUM_CLASSES = 1000
MARGIN = 0.75  # device blockmax fuzz bound vs exact f32 sim (bf16 inputs;
#                worst-case approx 0.33 input-quant + 0.35 bf16 out round;
#                measured max over 51M sims: 0.125)

_NC_CACHE = None


def _build_nc():
    nc = bass.Bass("TRN2", target_bir_lowering=False, debug=False,
                   num_devices=N_CORES)
    qT = nc.dram_tensor("qT", [D, B], mybir.dt.bfloat16,
                        kind="ExternalInput").ap()
    bankT = nc.dram_tensor("bankT", [D, NCOL], mybir.dt.bfloat16,
                           kind="ExternalInput").ap()
    bmA = nc.dram_tensor("bmA", [B, A_COLS], mybir.dt.bfloat16,
                         kind="ExternalOutput").ap()
    bmB = nc.dram_tensor("bmB", [B, B_COLS], mybir.dt.bfloat16,
                         kind="ExternalOutput").ap()

    with (
        nc.sbuf_tensor([D, B], mybir.dt.bfloat16) as qs,
        nc.sbuf_tensor([D, NCOL], mybir.dt.bfloat16) as banks,  # resident
        nc.psum_tensor([128, 2 * GW], mybir.dt.float32) as psum,  # ring 2
        nc.sbuf_tensor([128, 3 * AW_MAX], mybir.dt.bfloat16) as stage,
        nc.sbuf_tensor([128, 2 * (AW_MAX // 2)], mybir.dt.bfloat16) as l1,
        nc.sbuf_tensor([128, 2 * A_COLS], mybir.dt.bfloat16) as obufA,
        nc.sbuf_tensor([128, 2 * B_COLS], mybir.dt.bfloat16) as obufB,
        nc.semaphore() as warm_sem,
        nc.semaphore() as dma_sem,
        nc.semaphore() as st_sem,   # store-DMA completions (nobody waits)
        nc.semaphore() as mm0_sem,  # slot 0 cols [0:1536] done (ACT start)
        nc.semaphore() as mm_sem,   # +1 per slot (on its 4th matmul)
        nc.semaphore() as evacA,    # +1 per ACT slot copy
        nc.semaphore() as bsem,     # +1 per DVE direct reduce
        nc.semaphore() as fsem,     # +1 per DVE fold (slot fully done)
        nc.Block() as block,
    ):
        @block.sync
        def _(sync):
            sync.dma_start(qs[:], qT).then_inc(dma_sem, 16)
            for k in range(4):  # group 0 in quarters: earliest first matmul
                sync.dma_start(banks[:, k * 512:(k + 1) * 512],
                               bankT[:, k * 512:(k + 1) * 512]
                               ).then_inc(dma_sem, 16)
            for g in range(1, NBG):
                sync.dma_start(banks[:, g * GW:(g + 1) * GW],
                               bankT[:, g * GW:(g + 1) * GW]
                               ).then_inc(dma_sem, 16)
            # batched stores; thresholds count slots in j = 2g+c order
            a7, b7 = int(AOFF_G[7]), int(BOFF_G[7])
            a12, b12 = int(AOFF_G[12]), int(BOFF_G[12])
            stores = [
                (bmA, 0, 0, a7, fsem, 13), (bmA, 1, 0, a7, fsem, 14),
                (bmB, 0, 0, b7, bsem, 13), (bmB, 1, 0, b7, bsem, 14),
                (bmA, 0, a7, a12, fsem, 23), (bmA, 1, a7, a12, fsem, 24),
                (bmA, 0, a12, A_COLS, fsem, 25),
                (bmB, 0, b7, B_COLS, bsem, 25),
                (bmA, 1, a12, A_COLS, fsem, 26),
                (bmB, 1, b7, B_COLS, bsem, 26),
            ]
            for dram, c, lo, hi, sem, thr in stores:
                ob, w = (obufA, A_COLS) if dram is bmA else (obufB, B_COLS)
                sync.wait_ge(sem, thr)
                sync.dma_start(dram[c * 128:(c + 1) * 128, lo:hi],
                               ob[:, c * w + lo:c * w + hi]
                               ).then_inc(st_sem, 16)

        @block.tensor
        def _(tensor):
            for j in range(NSLOT):
                g, c = j // 2, j % 2
                s = (j % 2) * GW
                if j >= 2:  # psum ring: slot j-2 fully evacuated
                    tensor.wait_ge(evacA, j - 1)
                    tensor.wait_ge(bsem, j - 1)
                for k in range(4):
                    if g == 0 and c == 0:
                        tensor.wait_ge(dma_sem, 16 * (k + 2))
                    elif k == 0:
                        tensor.wait_ge(dma_sem, 16 * (g + 5))
                    mm = tensor.matmul(
                        psum[:, s + k * 512: s + (k + 1) * 512],
                        lhsT=qs[:, c * 128:(c + 1) * 128],
                        rhs=banks[:, g * GW + k * 512: g * GW + (k + 1) * 512],
                        start=True, stop=True)
                    if j == 0 and k == 2:
                        mm.then_inc(mm0_sem, 1)
                    if k == 3:
                        mm.then_inc(mm_sem, 1)

        @block.scalar
        def _(scalar):
            # warmup: load the ACT copy table before real work arrives
            scalar.wait_ge(warm_sem, 1)
            scalar.copy(stage[:, 0:1], stage[:, 1:2])
            for j in range(NSLOT):
                aw = AW_G[j // 2]
                if j >= 3:  # stage ring 3: fold of slot j-3 done
                    scalar.wait_ge(fsem, j - 2)
                if j == 0:
                    scalar.wait_ge(mm0_sem, 1)  # cols 0:1536 ready
                else:
                    scalar.wait_ge(mm_sem, j + 1)
                s = (j % 2) * GW
                ss = (j % 3) * AW_MAX
                scalar.copy(stage[:, ss:ss + aw],
                            psum[:, s:s + aw]).then_inc(evacA, 1)

        @block.vector
        def _(vector):
            MAX = mybir.AluOpType.max
            vector.memset(stage[:, 0:2], 0).then_inc(warm_sem, 1)
            for j in range(NSLOT):
                g, c = j // 2, j % 2
                aw = AW_G[g]
                ah = aw // 2
                aout = aw // 4
                bout = B_OUT_G[g]
                s = (j % 2) * GW
                ss = (j % 3) * AW_MAX
                ls = (j % 2) * (AW_MAX // 2)
                # direct blockmax-8 of psum cols [aw:GW]
                vector.wait_ge(mm_sem, j + 1)
                vector.tensor_reduce(
                    out=obufB[:, c * B_COLS + BOFF_G[g]:
                              c * B_COLS + BOFF_G[g] + bout],
                    in_=psum[:, s + aw:s + GW].rearrange(
                        "p (b w) -> p b w", w=8),
                    axis=mybir.AxisListType.X,
                    op=MAX,
                ).then_inc(bsem, 1)
                # 2-level fold tree of the staged A cols
                vector.wait_ge(evacA, j + 1)
                vector.tensor_tensor(
                    out=l1[:, ls:ls + ah], in0=stage[:, ss:ss + ah],
                    in1=stage[:, ss + ah:ss + aw], op=MAX)
                vector.tensor_tensor(
                    out=obufA[:, c * A_COLS + AOFF_G[g]:
                              c * A_COLS + AOFF_G[g] + aout],
                    in0=l1[:, ls:ls + aout],
                    in1=l1[:, ls + aout:ls + ah],
                    op=MAX).then_inc(fsem, 1)
    return nc


def _get_nc():
    global _NC_CACHE
    if _NC_CACHE is None:
        _NC_CACHE = _build_nc()
    return _NC_CACHE


def _run_device(query_feature, feature_bank, trace=False):
    qT = np.ascontiguousarray(query_feature.astype(np.float32).T
                              ).astype(BF16)  # [128, 256]
    fbT = feature_bank.astype(np.float32).T.astype(BF16)  # [D, N]
    in_maps = []
    for i in range(N_CORES):
        bt = np.zeros((D, NCOL), dtype=BF16)
        bt[:, :N_SHARD] = fbT[:, i * N_SHARD:(i + 1) * N_SHARD]
        in_maps.append({"qT": qT, "bankT": bt})
    nc = _get_nc()
    res = run_bass_kernel_spmd(nc, in_maps, list(range(N_CORES)), trace=trace)
    bmA = np.stack([res.results[i]["bmA"].astype(np.float32)
                    for i in range(N_CORES)])  # [8, 256, A_COLS]
    bmB = np.stack([res.results[i]["bmB"].astype(np.float32)
                    for i in range(N_CORES)])  # [8, 256, B_COLS]
    return (bmA, bmB), res


def _block_rows():
    """Local bank-row sets per (core-local) block, sentinel-padded to 8.

    Block order per core: A-blocks (AOFF_G layout) then B-blocks.
    A block (g, t) covers cols g*GW + {t, t+ao, t+2*ao, t+3*ao}, ao = aw/4;
    B block (g, u) covers cols g*GW + aw + 8u .. +8.
    Rows >= N_SHARD (padding) get sentinel (mapped to N_TOTAL later).
    """
    arows = []
    brows = []
    for g in range(NBG):
        aw = AW_G[g]
        ao = aw // 4
        t = np.arange(ao)[:, None]
        arows.append(g * GW + t + ao * np.arange(4)[None, :])
        u = np.arange(B_OUT_G[g])[:, None]
        brows.append(g * GW + aw + 8 * u + np.arange(8)[None, :])
    arows = np.concatenate(arows, axis=0)  # [A_COLS, 4]
    arows = np.concatenate(
        [arows, np.full((arows.shape[0], 4), N_SHARD)], axis=1)
    brows = np.concatenate(brows, axis=0)  # [B_COLS, 8]
    return np.concatenate([arows, brows], axis=0)  # [A_COLS+B_COLS, 8]


def _host_topk(bm, query_feature, feature_bank):
    """bm = (bmA, bmB): [8, 256, A_COLS], [8, 256, B_COLS] f32 device
    blockmaxima. Returns top-K indices [B, K] into the full bank, matching
    f32 jax top_k semantics."""
    bmA, bmB = bm
    q = query_feature.astype(np.float32)
    fb = feature_bank.astype(np.float32)

    lrows = _block_rows()  # [NBLK_CORE, 8] local rows (sentinel N_SHARD pad)
    nbc = lrows.shape[0]  # blocks per core
    grow = np.where(lrows < N_SHARD,
                    lrows[None, :, :] + (np.arange(N_CORES) * N_SHARD
                                         )[:, None, None],
                    N_TOTAL).reshape(-1, 8)  # [8*nbc, 8]

    bm_core = np.concatenate([bmA, bmB], axis=2)  # [8, 256, nbc]
    bm_flat = np.ascontiguousarray(
        bm_core.transpose(1, 0, 2).reshape(B, N_CORES * nbc))
    nblk = bm_flat.shape[1]

    # threshold: every block whose blockmax could reach the top-K values
    bm200 = -np.partition(-bm_flat, K - 1, axis=1)[:, K - 1]
    nsel = (bm_flat >= (bm200 - 2 * MARGIN)[:, None]).sum(axis=1)
    nb = int(nsel.max())

    topk_idx = np.empty((B, K), dtype=np.int64)
    pending = np.arange(B)
    while len(pending):
        nb = min(nb, nblk)
        part = np.argpartition(-bm_flat[pending], nb - 1, axis=1)
        still = []
        for chunk in range(0, len(pending), 64):
            pc = pending[chunk:chunk + 64]
            nq = len(pc)
            sel = part[chunk:chunk + 64, :nb]
            # gather A-blocks (4 real rows) and B-blocks (8) separately
            pairs = sel.ravel()
            pair_q = np.repeat(np.arange(nq), nb)
            isA = (pairs % nbc) < A_COLS
            sims8 = np.full((nq * nb, 8), -np.inf, dtype=np.float32)
            rows8 = grow[pairs]
            for msk, w in ((isA, 4), (~isA, 8)):
                r = rows8[msk, :w]
                s = np.einsum("prd,pd->pr", fb[np.minimum(r, N_TOTAL - 1)],
                              q[pc][pair_q[msk]], optimize=True)
                s[r == N_TOTAL] = -np.inf
                sims8[msk, :w] = s
            sims = sims8.reshape(nq, nb * 8)
            rows = rows8.reshape(nq, nb * 8)
            for i, b in enumerate(pc):
                o = np.lexsort((rows[i], -sims[i]))[:K]
                tK = sims[i][o[-1]]
                if nb >= nblk:
                    topk_idx[b] = rows[i][o]
                    continue
                unsel = bm_flat[b][part[chunk + i, nb:]].max()
                if unsel + MARGIN < tK:
                    topk_idx[b] = rows[i][o]
                else:
                    still.append(b)
        pending = np.array(still, dtype=np.int64)
        nb *= 2
    return topk_idx


def _labels_to_output(topk_idx, target_bank):
    tb = np.asarray(target_bank).astype(np.int64)
    mask = np.zeros((B, NUM_CLASSES), dtype=bool)
    mask[np.arange(B)[:, None], tb[topk_idx]] = True
    # voted classes ascending, then unvoted ascending
    return np.argsort(~mask, axis=1, kind="stable").astype(np.int32)


def kernel(query_feature, feature_bank, target_bank):
    query_feature = np.asarray(query_feature)
    feature_bank = np.asarray(feature_bank)
    target_bank = np.asarray(target_bank)
    bm, _ = _run_device(query_feature, feature_bank)
    topk_idx = _host_topk(bm, query_feature, feature_bank)
    return _labels_to_output(topk_idx, target_bank)


# revision 36
# speedup vs baseline: 1.0220x; 1.0058x over previous
"""Distributed KNN online evaluator kernel for 8 trn2 NeuronCores.

Device side (SPMD over 8 cores, bank sharded over N):
  - bf16 matmul sim tiles (queries stationary) -> f32 PSUM, 26 slots of
    2048 cols (13 bank groups x 2 query chunks), psum ring 2
  - split-slot evac, engine-balanced, all PSUM-bank aligned:
      ACT: copy psum cols [0:1536] -> bf16 stage           (~1465 ns)
      DVE: blockmax-8 reduce of psum cols [1536:2048]      ( ~659 ns)
           + 2-level TT fold tree of the staged 1536       ( ~720 ns)
    (mid-bank reduce starts crash the HW; offsets must stay bank aligned)
  - bank fully resident in SBUF (53KB/partition), loaded once
  - batched blockmax stores (8 DMAs)

Host side:
  - one-shot drill-down: select blocks whose blockmax could contain a
    global top-K sim, recompute those sims exactly in f32, take top-K;
    verified sound (unselected blocks provably below the top-K threshold
    within MARGIN) with an expansion fallback loop
  - class votes with inf weights degenerate to membership -> output is
    [voted classes asc, unvoted classes asc] per query
"""

import numpy as np
import ml_dtypes

import concourse.bass as bass
import concourse.mybir as mybir
from concourse.bass_utils import run_bass_kernel_spmd

BF16 = ml_dtypes.bfloat16

N_CORES = 8
B = 256  # queries
D = 128  # feature dim
N_TOTAL = 200000
N_SHARD = N_TOTAL // N_CORES  # 25000
NBG = 13  # bank groups
GW = 2048  # group width (cols) = one psum slot
NCOL = NBG * GW  # padded shard width 26624
NSLOT = 2 * NBG  # 26 compute slots: slot j -> group j//2, chunk j%2
S_GROUPS = set()  # groups with a 1024/1024 split; empty: uniform 1536/512
#                   (DVE-heavy slots stall the 2-deep psum ring, so uniform
#                   ACT-leaning slots win despite higher ACT busy)
AW_G = [1024 if g in S_GROUPS else 1536 for g in range(NBG)]
A_OUT_G = [aw // 4 for aw in AW_G]  # A-blocks per group (stride aw/4)
B_OUT_G = [(GW - aw) // 8 for aw in AW_G]  # B-blocks per group
AOFF_G = np.concatenate([[0], np.cumsum(A_OUT_G)]).astype(int)
BOFF_G = np.concatenate([[0], np.cumsum(B_OUT_G)]).astype(int)
A_COLS = int(AOFF_G[-1])  # 4992 per chunk
B_COLS = int(BOFF_G[-1])  # 832 per chunk
AW_MAX = 1536
K = 200
NUM_CLASSES = 1000
MARGIN = 0.75  # device blockmax fuzz bound vs exact f32 sim (bf16 inputs;
#                worst-case approx 0.33 input-quant + 0.35 bf16 out round;
#                measured max over 51M sims: 0.125)

_NC_CACHE = None


def _build_nc():
    nc = bass.Bass("TRN2", target_bir_lowering=False, debug=False,
                   num_devices=N_CORES)
    qT = nc.dram_tensor("qT", [D, B], mybir.dt.bfloat16,
                        kind="ExternalInput").ap()
    bankT = nc.dram_tensor("bankT", [D, NCOL], mybir.dt.bfloat16,
                           kind="ExternalInput").ap()
    bmA = nc.dram_tensor("bmA", [B, A_COLS], mybir.dt.bfloat16,
                         kind="ExternalOutput").ap()
    bmB = nc.dram_tensor("bmB", [B, B_COLS], mybir.dt.bfloat16,
                         kind="ExternalOutput").ap()

    with (
        nc.sbuf_tensor([D, B], mybir.dt.bfloat16) as qs,
        nc.sbuf_tensor([D, NCOL], mybir.dt.bfloat16) as banks,  # resident
        nc.psum_tensor([128, 2 * GW], mybir.dt.float32) as psum,  # ring 2
        nc.sbuf_tensor([128, 3 * AW_MAX], mybir.dt.bfloat16) as stage,
        nc.sbuf_tensor([128, 2 * (AW_MAX // 2)], mybir.dt.bfloat16) as l1,
        nc.sbuf_tensor([128, 2 * A_COLS], mybir.dt.bfloat16) as obufA,
        nc.sbuf_tensor([128, 2 * B_COLS], mybir.dt.bfloat16) as obufB,
        nc.semaphore() as warm_sem,
        nc.semaphore() as dma_sem,
        nc.semaphore() as st_sem,   # store-DMA completions (nobody waits)
        nc.semaphore() as mm0_sem,  # slot 0 cols [0:1536] done (ACT start)
        nc.semaphore() as mm_sem,   # +1 per slot (on its 4th matmul)
        nc.semaphore() as evacA,    # +1 per ACT slot copy
        nc.semaphore() as bsem,     # +1 per DVE direct reduce
        nc.semaphore() as fsem,     # +1 per DVE fold (slot fully done)
        nc.Block() as block,
    ):
        @block.sync
        def _(sync):
            sync.dma_start(qs[:], qT).then_inc(dma_sem, 16)
            for k in range(4):  # group 0 in quarters: earliest first matmul
                sync.dma_start(banks[:, k * 512:(k + 1) * 512],
                               bankT[:, k * 512:(k + 1) * 512]
                               ).then_inc(dma_sem, 16)
            for g in range(1, NBG):
                sync.dma_start(banks[:, g * GW:(g + 1) * GW],
                               bankT[:, g * GW:(g + 1) * GW]
                               ).then_inc(dma_sem, 16)
            # batched stores; thresholds count slots in j = 2g+c order
            a7, b7 = int(AOFF_G[7]), int(BOFF_G[7])
            a12, b12 = int(AOFF_G[12]), int(BOFF_G[12])
            stores = [
                (bmA, 0, 0, a7, fsem, 13), (bmA, 1, 0, a7, fsem, 14),
                (bmB, 0, 0, b7, bsem, 13), (bmB, 1, 0, b7, bsem, 14),
                (bmA, 0, a7, a12, fsem, 23), (bmA, 1, a7, a12, fsem, 24),
                (bmA, 0, a12, A_COLS, fsem, 25),
                (bmB, 0, b7, B_COLS, bsem, 25),
                (bmA, 1, a12, A_COLS, fsem, 26),
                (bmB, 1, b7, B_COLS, bsem, 26),
            ]
            for dram, c, lo, hi, sem, thr in stores:
                ob, w = (obufA, A_COLS) if dram is bmA else (obufB, B_COLS)
                sync.wait_ge(sem, thr)
                sync.dma_start(dram[c * 128:(c + 1) * 128, lo:hi],
                               ob[:, c * w + lo:c * w + hi]
                               ).then_inc(st_sem, 16)

        @block.tensor
        def _(tensor):
            for j in range(NSLOT):
                g, c = j // 2, j % 2
                s = (j % 2) * GW
                if j >= 2:  # psum ring: slot j-2 fully evacuated
                    tensor.wait_ge(evacA, j - 1)
                    tensor.wait_ge(bsem, j - 1)
                for k in range(4):
                    if g == 0 and c == 0:
                        tensor.wait_ge(dma_sem, 16 * (k + 2))
                    elif k == 0:
                        tensor.wait_ge(dma_sem, 16 * (g + 5))
                    mm = tensor.matmul(
                        psum[:, s + k * 512: s + (k + 1) * 512],
                        lhsT=qs[:, c * 128:(c + 1) * 128],
                        rhs=banks[:, g * GW + k * 512: g * GW + (k + 1) * 512],
                        start=True, stop=True)
                    if j == 0 and k < 3:
                        mm.then_inc(mm0_sem, 1)
                    if k == 3:
                        mm.then_inc(mm_sem, 1)

        @block.scalar
        def _(scalar):
            # warmup: load the ACT copy table before real work arrives
            scalar.wait_ge(warm_sem, 1)
            scalar.copy(stage[:, 0:1], stage[:, 1:2])
            for j in range(NSLOT):
                aw = AW_G[j // 2]
                if j >= 3:  # stage ring 3: fold of slot j-3 done
                    scalar.wait_ge(fsem, j - 2)
                s = (j % 2) * GW
                ss = (j % 3) * AW_MAX
                if j == 0:
                    # chase the cold matmuls in 512-col pieces
                    for p in range(3):
                        scalar.wait_ge(mm0_sem, p + 1)
                        cp = scalar.copy(
                            stage[:, ss + p * 512:ss + (p + 1) * 512],
                            psum[:, s + p * 512:s + (p + 1) * 512])
                        if p == 2:
                            cp.then_inc(evacA, 1)
                    continue
                scalar.wait_ge(mm_sem, j + 1)
                scalar.copy(stage[:, ss:ss + aw],
                            psum[:, s:s + aw]).then_inc(evacA, 1)

        @block.vector
        def _(vector):
            MAX = mybir.AluOpType.max
            vector.memset(stage[:, 0:2], 0).then_inc(warm_sem, 1)
            for j in range(NSLOT):
                g, c = j // 2, j % 2
                aw = AW_G[g]
                ah = aw // 2
                aout = aw // 4
                bout = B_OUT_G[g]
                s = (j % 2) * GW
                ss = (j % 3) * AW_MAX
                ls = (j % 2) * (AW_MAX // 2)
                # direct blockmax-8 of psum cols [aw:GW]
                vector.wait_ge(mm_sem, j + 1)
                vector.tensor_reduce(
                    out=obufB[:, c * B_COLS + BOFF_G[g]:
                              c * B_COLS + BOFF_G[g] + bout],
                    in_=psum[:, s + aw:s + GW].rearrange(
                        "p (b w) -> p b w", w=8),
                    axis=mybir.AxisListType.X,
                    op=MAX,
                ).then_inc(bsem, 1)
                # 2-level fold tree of the staged A cols
                vector.wait_ge(evacA, j + 1)
                vector.tensor_tensor(
                    out=l1[:, ls:ls + ah], in0=stage[:, ss:ss + ah],
                    in1=stage[:, ss + ah:ss + aw], op=MAX)
                vector.tensor_tensor(
                    out=obufA[:, c * A_COLS + AOFF_G[g]:
                              c * A_COLS + AOFF_G[g] + aout],
                    in0=l1[:, ls:ls + aout],
                    in1=l1[:, ls + aout:ls + ah],
                    op=MAX).then_inc(fsem, 1)
    return nc


def _get_nc():
    global _NC_CACHE
    if _NC_CACHE is None:
        _NC_CACHE = _build_nc()
    return _NC_CACHE


def _run_device(query_feature, feature_bank, trace=False):
    qT = np.ascontiguousarray(query_feature.astype(np.float32).T
                              ).astype(BF16)  # [128, 256]
    fbT = feature_bank.astype(np.float32).T.astype(BF16)  # [D, N]
    in_maps = []
    for i in range(N_CORES):
        bt = np.zeros((D, NCOL), dtype=BF16)
        bt[:, :N_SHARD] = fbT[:, i * N_SHARD:(i + 1) * N_SHARD]
        in_maps.append({"qT": qT, "bankT": bt})
    nc = _get_nc()
    res = run_bass_kernel_spmd(nc, in_maps, list(range(N_CORES)), trace=trace)
    bmA = np.stack([res.results[i]["bmA"].astype(np.float32)
                    for i in range(N_CORES)])  # [8, 256, A_COLS]
    bmB = np.stack([res.results[i]["bmB"].astype(np.float32)
                    for i in range(N_CORES)])  # [8, 256, B_COLS]
    return (bmA, bmB), res


def _block_rows():
    """Local bank-row sets per (core-local) block, sentinel-padded to 8.

    Block order per core: A-blocks (AOFF_G layout) then B-blocks.
    A block (g, t) covers cols g*GW + {t, t+ao, t+2*ao, t+3*ao}, ao = aw/4;
    B block (g, u) covers cols g*GW + aw + 8u .. +8.
    Rows >= N_SHARD (padding) get sentinel (mapped to N_TOTAL later).
    """
    arows = []
    brows = []
    for g in range(NBG):
        aw = AW_G[g]
        ao = aw // 4
        t = np.arange(ao)[:, None]
        arows.append(g * GW + t + ao * np.arange(4)[None, :])
        u = np.arange(B_OUT_G[g])[:, None]
        brows.append(g * GW + aw + 8 * u + np.arange(8)[None, :])
    arows = np.concatenate(arows, axis=0)  # [A_COLS, 4]
    arows = np.concatenate(
        [arows, np.full((arows.shape[0], 4), N_SHARD)], axis=1)
    brows = np.concatenate(brows, axis=0)  # [B_COLS, 8]
    return np.concatenate([arows, brows], axis=0)  # [A_COLS+B_COLS, 8]


def _host_topk(bm, query_feature, feature_bank):
    """bm = (bmA, bmB): [8, 256, A_COLS], [8, 256, B_COLS] f32 device
    blockmaxima. Returns top-K indices [B, K] into the full bank, matching
    f32 jax top_k semantics."""
    bmA, bmB = bm
    q = query_feature.astype(np.float32)
    fb = feature_bank.astype(np.float32)

    lrows = _block_rows()  # [NBLK_CORE, 8] local rows (sentinel N_SHARD pad)
    nbc = lrows.shape[0]  # blocks per core
    grow = np.where(lrows < N_SHARD,
                    lrows[None, :, :] + (np.arange(N_CORES) * N_SHARD
                                         )[:, None, None],
                    N_TOTAL).reshape(-1, 8)  # [8*nbc, 8]

    bm_core = np.concatenate([bmA, bmB], axis=2)  # [8, 256, nbc]
    bm_flat = np.ascontiguousarray(
        bm_core.transpose(1, 0, 2).reshape(B, N_CORES * nbc))
    nblk = bm_flat.shape[1]

    # threshold: every block whose blockmax could reach the top-K values
    bm200 = -np.partition(-bm_flat, K - 1, axis=1)[:, K - 1]
    nsel = (bm_flat >= (bm200 - 2 * MARGIN)[:, None]).sum(axis=1)
    nb = int(nsel.max())

    topk_idx = np.empty((B, K), dtype=np.int64)
    pending = np.arange(B)
    while len(pending):
        nb = min(nb, nblk)
        part = np.argpartition(-bm_flat[pending], nb - 1, axis=1)
        still = []
        for chunk in range(0, len(pending), 64):
            pc = pending[chunk:chunk + 64]
            nq = len(pc)
            sel = part[chunk:chunk + 64, :nb]
            # gather A-blocks (4 real rows) and B-blocks (8) separately
            pairs = sel.ravel()
            pair_q = np.repeat(np.arange(nq), nb)
            isA = (pairs % nbc) < A_COLS
            sims8 = np.full((nq * nb, 8), -np.inf, dtype=np.float32)
            rows8 = grow[pairs]
            for msk, w in ((isA, 4), (~isA, 8)):
                r = rows8[msk, :w]
                s = np.einsum("prd,pd->pr", fb[np.minimum(r, N_TOTAL - 1)],
                              q[pc][pair_q[msk]], optimize=True)
                s[r == N_TOTAL] = -np.inf
                sims8[msk, :w] = s
            sims = sims8.reshape(nq, nb * 8)
            rows = rows8.reshape(nq, nb * 8)
            for i, b in enumerate(pc):
                o = np.lexsort((rows[i], -sims[i]))[:K]
                tK = sims[i][o[-1]]
                if nb >= nblk:
                    topk_idx[b] = rows[i][o]
                    continue
                unsel = bm_flat[b][part[chunk + i, nb:]].max()
                if unsel + MARGIN < tK:
                    topk_idx[b] = rows[i][o]
                else:
                    still.append(b)
        pending = np.array(still, dtype=np.int64)
        nb *= 2
    return topk_idx


def _labels_to_output(topk_idx, target_bank):
    tb = np.asarray(target_bank).astype(np.int64)
    mask = np.zeros((B, NUM_CLASSES), dtype=bool)
    mask[np.arange(B)[:, None], tb[topk_idx]] = True
    # voted classes ascending, then unvoted ascending
    return np.argsort(~mask, axis=1, kind="stable").astype(np.int32)


def kernel(query_feature, feature_bank, target_bank):
    query_feature = np.asarray(query_feature)
    feature_bank = np.asarray(feature_bank)
    target_bank = np.asarray(target_bank)
    bm, _ = _run_device(query_feature, feature_bank)
    topk_idx = _host_topk(bm, query_feature, feature_bank)
    return _labels_to_output(topk_idx, target_bank)
